# revision 28
# baseline (speedup 1.0000x reference)
"""Trainium2 Bass kernel for the fused attention block
(QKV projection + RMSNorm + 3D RoPE + softmax attention + output projection),
tensor-parallel over heads across 8 NeuronCores.

Sharding: 3 heads per core. Each core computes its heads' QKV columns
(column-parallel), runs attention for (3 heads x 2 batches), and produces a
row-parallel partial of the output projection; the host sums the 8 partials
and adds proj_b (plus the folded v-bias term: attn rows sum to 1, so
attn@(v+bv) = attn@v + bv, and bv @ proj_w.T is a constant added on host).

v2 design vs the spill-based baseline:
 - bf16 data plane end-to-end (x, w, q/k/v, attention operands, proj weights,
   output partials); matmuls accumulate in fp32 PSUM, softmax/norm arithmetic
   in fp32.
 - q/k/v stay resident in SBUF (9.4 MB bf16); no DRAM spill round trip.
 - q/k transposed via the DMA XBAR (dma_start_transpose, 14ns/tile) instead
   of PE identity matmuls; PE does zero transpose work.
 - Phase B restructured per (batch, 512-query span): attention for all 3
   heads then immediately the output projection of that span, so the
   projection pipeline drains alongside attention instead of in a tail.
 - 1/den broadcast via on-chip partition_broadcast (no DRAM round trip).
"""
import sys
sys.path.insert(0, '/opt/trn_rl_repo')

import numpy as np
import concourse.bass as bass
import concourse.mybir as mybir
import concourse.tile as tile
from concourse import bacc
from concourse.bass import ts, ds

F32 = mybir.dt.float32
F32R = mybir.dt.float32r
FP8 = mybir.dt.float8e4
BF16 = mybir.dt.bfloat16
AF = mybir.ActivationFunctionType
P = 128


class Cfg:
    def __init__(self, NB=2048, CIN=3072, COUT=3072, HPC=3, B=2, D=128,
                 eps=1e-6, loop_iters=1, rope_shared=True, newton_iters=3,
                 xt_bufs=3, stps_bufs=2, outps_bufs=1, pexp_bufs=10,
                 pcps_bufs=2, SPAN=512, pe_transpose=False, dma_split=True,
                 fp8_den=False, act_rsqrt=True):
        self.__dict__.update(locals())
        del self.__dict__['self']
        self.TOK = B * NB
        self.KC = CIN // P           # contraction chunks for qkv matmul
        self.G = HPC * D             # per-core head channels (per q/k/v)
        self.TT = self.TOK // P      # token tiles total
        self.TPB = NB // P           # token tiles per batch
        self.NSPAN = NB // SPAN      # query spans per batch
        self.KCH = NB // P           # key chunks per batch
        self.OC = 512
        self.NOC = COUT // self.OC
        self.TPS = SPAN // P         # token tiles per span
        assert NB % SPAN == 0 and CIN % P == 0 and COUT % self.OC == 0


def _phase_a(nc, tc, c, io, sc, res):
    """QKV matmul + bias + rms-norm + rope; q/k into SBUF transposed via the
    DMA XBAR, v into SBUF in [tok, d] layout. All bf16."""
    D, G, KC, TT, TPB, HPC = c.D, c.G, c.KC, c.TT, c.TPB, c.HPC
    MUL, ADD = mybir.AluOpType.mult, mybir.AluOpType.add
    qdma = nc.scalar if c.dma_split else nc.sync
    with tc.tile_pool(name="paconst", bufs=1) as paconst, \
         tc.tile_pool(name="pa", bufs=2) as pa, \
         tc.tile_pool(name="paps", bufs=2, space="PSUM") as paps, \
         tc.tile_pool(name="tpps", bufs=2, space="PSUM") as tpps:
        xT3 = io['xT'].rearrange("(kc p) n -> p kc n", p=P)
        wT3 = io['wT'].rearrange("(kc p) g -> p kc g", p=P)
        # x tiles stream on the SP queue; weights/tables on the ACT queue so
        # the first matmuls aren't stuck behind the 7MB weight load
        xt0 = pa.tile([P, KC, P], BF16, tag="xt", name="xt_0", bufs=c.xt_bufs)
        nc.sync.dma_start(xt0, xT3[:, :, ts(0, P)])
        w_sb = paconst.tile([P, KC, 3 * G], BF16)
        for kc in range(KC):
            qdma.dma_start(w_sb[:, ds(kc, 1)], wT3[:, ds(kc, 1)])
        if c.pe_transpose:
            from concourse.masks import make_identity
            ident = paconst.tile([P, P], BF16, name="ident")
            make_identity(nc, ident)
        bias_sb = paconst.tile([P, 2 * G], F32)
        nc.gpsimd.dma_start(bias_sb, io['bias'].partition_broadcast(P))
        if c.rope_shared:
            cos_t = paconst.tile([P, TPB, D], F32, name="cos_t")
            sin_t = paconst.tile([P, TPB, D], F32, name="sin_t")
            qdma.dma_start(cos_t, io['cosq'].rearrange("(tt p) d -> p tt d", p=P))
            qdma.dma_start(sin_t, io['sinq'].rearrange("(tt p) d -> p tt d", p=P))
            cos_sb = {"q": cos_t, "k": cos_t}
            sin_sb = {"q": sin_t, "k": sin_t}
        else:
            cos_sb, sin_sb = {}, {}
            for nm in ("q", "k"):
                ct = paconst.tile([P, TPB, D], F32, tag=f"cos_{nm}", name=f"cos_{nm}")
                st2 = paconst.tile([P, TPB, D], F32, tag=f"sin_{nm}", name=f"sin_{nm}")
                qdma.dma_start(ct, io[f'cos{nm}'].rearrange("(tt p) d -> p tt d", p=P))
                qdma.dma_start(st2, io[f'sin{nm}'].rearrange("(tt p) d -> p tt d", p=P))
                cos_sb[nm], sin_sb[nm] = ct, st2
        for t in range(TT):
            b, tb = t // TPB, t % TPB
            if t == 0:
                xt = xt0
            else:
                xt = pa.tile([P, KC, P], BF16, tag="xt", name=f"xt_{t}",
                             bufs=c.xt_bufs)
                nc.sync.dma_start(xt, xT3[:, :, ts(t, P)])
            ps = {}
            for s, name in enumerate(("q", "k", "v")):
                ps[name] = paps.tile([P, G], F32, tag=f"ps_{name}",
                                     name=f"ps_{name}_{t}", bufs=2)
            for kc in range(KC):
                for s, name in enumerate(("q", "k", "v")):
                    nc.tensor.matmul(ps[name], xt[:, kc],
                                     w_sb[:, kc, ds(s * G, G)],
                                     start=(kc == 0), stop=(kc == KC - 1))
            # v: PSUM -> resident SBUF bf16 (bias folded into host proj_b);
            # Pool can't read PSUM, so this goes on ACT
            nc.scalar.copy(res['v'][b][:, ds(tb, 1)],
                           ps["v"].rearrange("p (o g) -> p o g", o=1))
            for si, name in enumerate(("q", "k")):
                raw = pa.tile([P, G], F32, tag="raw")
                nc.vector.tensor_add(raw, ps[name], bias_sb[:, ds(si * G, G)])
                raw3 = raw.rearrange("p (h d) -> p h d", d=D)
                # per-head sum of squares on ACT
                ssum = pa.tile([P, HPC], F32, tag="ssum")
                sqscr = pa.tile([P, D], F32, tag="sqscr")
                for h in range(HPC):
                    nc.scalar.activation(sqscr, raw3[:, h], AF.Square,
                                         accum_out=ssum[:, ds(h, 1)])
                # m = ssum/D + eps; rstd = 1/sqrt(m): ACT sqrt (same table
                # set as Square) + DVE reciprocal - 2 ops instead of Newton
                m_t = pa.tile([P, HPC], F32, tag="m_t")
                nc.vector.tensor_scalar(m_t, ssum, 1.0 / D, c.eps, MUL, ADD)
                rstd = pa.tile([P, HPC], F32, tag="rstd")
                if c.act_rsqrt:
                    sq_m = pa.tile([P, HPC], F32, tag="sq_m")
                    nc.scalar.activation(sq_m, m_t, AF.Sqrt)
                    nc.vector.reciprocal(rstd, sq_m)
                else:
                    nc.vector.tensor_scalar(rstd, m_t, -0.5, 1.5, MUL, ADD)
                    nt1 = pa.tile([P, HPC], F32, tag="nt1")
                    for _ in range(c.newton_iters):
                        nc.vector.tensor_mul(nt1, rstd, rstd)
                        nc.vector.tensor_mul(nt1, nt1, m_t)
                        nc.vector.tensor_scalar(nt1, nt1, -0.5, 1.5, MUL, ADD)
                        nc.vector.tensor_mul(rstd, rstd, nt1)
                # pair swap on Pool (sw[2i]=raw[2i+1], sw[2i+1]=raw[2i])
                sw = pa.tile([P, HPC, D], F32, tag="sw")
                raw2 = raw.rearrange("p (a two) -> p a two", two=2)
                sw2 = sw.rearrange("p h (a two) -> p (h a) two", two=2)
                nc.gpsimd.tensor_copy(sw2[:, :, ds(0, 1)], raw2[:, :, ds(1, 1)])
                nc.gpsimd.tensor_copy(sw2[:, :, ds(1, 1)], raw2[:, :, ds(0, 1)])
                # rope fused with rstd apply: ro = (raw*rstd)*cosW + (sw*rstd)*sinW
                ro = pa.tile([P, HPC, D], BF16, tag=f"ro_{name}")
                rtmp = pa.tile([P, HPC, D], F32, tag="rtmp")
                rtmp2 = pa.tile([P, HPC, D], F32, tag="rtmp2")
                for h in range(HPC):
                    nc.vector.scalar_tensor_tensor(
                        rtmp[:, h], sw[:, h], rstd[:, ds(h, 1)],
                        sin_sb[name][:, tb], MUL, MUL)
                    nc.vector.scalar_tensor_tensor(
                        rtmp2[:, h], raw3[:, h], rstd[:, ds(h, 1)],
                        cos_sb[name][:, tb], MUL, MUL)
                    nc.vector.tensor_add(ro[:, h], rtmp2[:, h], rtmp[:, h])
                dst = res['qT'] if name == "q" else res['kT']
                for h in range(HPC):
                    if c.pe_transpose:
                        tp = tpps.tile([P, P], BF16, tag="tp")
                        nc.tensor.matmul(tp, ro[:, h], ident, is_transpose=True)
                        nc.vector.tensor_copy(dst[b][:, h, ts(tb, P)], tp)
                    else:
                        nc.sync.dma_start_transpose(dst[b][:, h, ts(tb, P)], ro[:, h])


def _phase_bc(nc, tc, c, io, sc):
    """Per (batch, query-span): attention for all heads, then that span's
    output projection."""
    D, HPC, B = c.D, c.HPC, c.B
    NB, SPAN, NSPAN, KCH = c.NB, c.SPAN, c.NSPAN, c.KCH
    NOC, OC, COUT, TPS = c.NOC, c.OC, c.COUT, c.TPS
    res = io['res']
    scale = float(D) ** -0.5
    with tc.tile_pool(name="pb", bufs=2) as pb, \
         tc.tile_pool(name="pbc", bufs=1) as pbc, \
         tc.tile_pool(name="pexpp", bufs=c.pexp_bufs) as pexpp, \
         tc.tile_pool(name="pexp8p", bufs=2) as pexp8p, \
         tc.tile_pool(name="stps", bufs=c.stps_bufs, space="PSUM") as stps, \
         tc.tile_pool(name="outps", bufs=c.outps_bufs, space="PSUM") as outps, \
         tc.tile_pool(name="denps", bufs=1, space="PSUM") as denps, \
         tc.tile_pool(name="pcps", bufs=c.pcps_bufs, space="PSUM") as pcps:
        pw_sb = pbc.tile([P, HPC, COUT], BF16, bufs=1)
        pwT3 = io['pwT'].rearrange("(h p) o -> p h o", p=P)
        qdma = nc.scalar if c.dma_split else nc.sync
        for h in range(HPC):
            qdma.dma_start(pw_sb[:, ds(h, 1)], pwT3[:, ds(h, 1)])
        for b in range(B):
            for s in range(NSPAN):
                at = {}
                for h in range(HPC):
                    qspan = res['qT'][b][:, h, ds(s * SPAN, SPAN)]
                    pexps = []
                    pexp8s = []
                    for pr in range(KCH // 2):
                        stp = stps.tile([P, 2 * SPAN], F32, tag="stp")
                        for j in (0, 1):
                            kc = 2 * pr + j
                            nc.tensor.matmul(stp[:, ds(j * SPAN, SPAN)],
                                             res['kT'][b][:, h, ts(kc, P)],
                                             qspan, start=True, stop=True)
                        pexp = pexpp.tile([P, 2 * SPAN], BF16, tag="pexp")
                        nc.scalar.activation(pexp, stp, AF.Exp,
                                             bias=sc['gate'], scale=scale)
                        pexps.append(pexp)
                        if pr == 0:
                            outp = outps.tile([P, SPAN], F32, tag="outp")
                        for j in (0, 1):
                            kc = 2 * pr + j
                            nc.tensor.matmul(outp, res['v'][b][:, kc, ds(h * D, D)],
                                             pexp[:, ds(j * SPAN, SPAN)],
                                             start=(kc == 0), stop=(kc == KCH - 1))
                        if c.fp8_den:
                            pexp8 = pexp8p.tile([P, 2, SPAN], FP8, tag="pexp8",
                                                bufs=10)
                            if pr % 2 == 0:
                                nc.gpsimd.tensor_copy(
                                    pexp8, pexp.rearrange("p (two s) -> p two s", two=2))
                            else:
                                nc.vector.tensor_copy(
                                    pexp8, pexp.rearrange("p (two s) -> p two s", two=2))
                            pexp8s.append(pexp8)
                    # den with an all-ones stationary: every output partition
                    # gets the key-sum, so no broadcast is needed
                    denp = denps.tile([P, SPAN], F32, tag="denp")
                    if c.fp8_den:
                        for pr in range(KCH // 2):
                            nc.tensor.matmul(denp, sc['ones8'], pexp8s[pr],
                                             start=(pr == 0),
                                             stop=(pr == KCH // 2 - 1),
                                             perf_mode=mybir.MatmulPerfMode.DoubleRow)
                    else:
                        for pr in range(KCH // 2):
                            for j in (0, 1):
                                kc = 2 * pr + j
                                nc.tensor.matmul(denp, sc['ones_sq'],
                                                 pexps[pr][:, ds(j * SPAN, SPAN)],
                                                 start=(kc == 0), stop=(kc == KCH - 1))
                    drep = pb.tile([P, SPAN], F32, tag="drep")
                    nc.vector.reciprocal(drep, denp)
                    ath = pb.tile([P, SPAN], BF16, tag=f"at_{h}")
                    nc.vector.tensor_mul(ath, outp, drep)
                    at[h] = ath
                # ---- projection for this span ----
                for tb in range(TPS):
                    t = b * (NB // P) + s * TPS + tb
                    for o in range(NOC):
                        op = pcps.tile([P, OC], F32, tag="op", name=f"op_{t}_{o}")
                        for h in range(HPC):
                            nc.tensor.matmul(op, at[h][:, ts(tb, P)],
                                             pw_sb[:, h, ds(o * OC, OC)],
                                             start=(h == 0), stop=(h == HPC - 1))
                        ost = pbc.tile([P, OC], BF16, tag="ost",
                                       name=f"ost_{t}_{o}", bufs=6)
                        # Pool can't read PSUM: rotate DVE-heavy (bf16 out
                        # runs 2x on DVE) with ACT taking every third
                        if (tb * NOC + o) % 3 == 2:
                            nc.scalar.copy(ost, op)
                        else:
                            nc.vector.tensor_copy(ost, op)
                        nc.sync.dma_start(io['out_part'][ts(t, P), ds(o * OC, OC)],
                                          ost)


def build_program(**kw):
    c = Cfg(**kw)
    nc = bacc.Bacc("TRN2", target_bir_lowering=False, debug=False,
                   enable_asserts=False, num_devices=8)

    io = {}
    io['xT'] = nc.dram_tensor("xT", [c.CIN, c.TOK], BF16, kind="ExternalInput").ap()
    io['wT'] = nc.dram_tensor("wT", [c.CIN, 3 * c.G], BF16, kind="ExternalInput").ap()
    io['bias'] = nc.dram_tensor("bias", [2 * c.G], F32, kind="ExternalInput").ap()
    for nm in ("q", "k"):
        io[f'cos{nm}'] = nc.dram_tensor(f"cos{nm}", [c.NB, c.D], F32,
                                        kind="ExternalInput").ap()
        io[f'sin{nm}'] = nc.dram_tensor(f"sin{nm}", [c.NB, c.D], F32,
                                        kind="ExternalInput").ap()
    io['pwT'] = nc.dram_tensor("pwT", [c.G, c.COUT], BF16, kind="ExternalInput").ap()
    io['out_part'] = nc.dram_tensor("out_part", [c.TOK, c.COUT], BF16,
                                    kind="ExternalOutput").ap()

    with tile.TileContext(nc) as tc:
        with tc.tile_pool(name="const", bufs=1) as constp, \
             tc.tile_pool(name="resp", bufs=1) as resp:
            sc = {}
            ones_sq_f = constp.tile([P, P], F32)
            nc.vector.memset(ones_sq_f, 1.0)
            ones_sq = constp.tile([P, P], BF16)
            nc.vector.tensor_copy(ones_sq, ones_sq_f)
            ones8_f = constp.tile([P, 2, P], F32)
            nc.vector.memset(ones8_f, 1.0)
            ones8 = constp.tile([P, 2, P], FP8)
            nc.vector.tensor_copy(ones8, ones8_f)
            gate = constp.tile([P, 1], F32)
            nc.vector.memset(gate, 0.0)
            sc.update(ones_sq=ones_sq, ones8=ones8, gate=gate)

            res = {'qT': {}, 'kT': {}, 'v': {}}
            for b in range(c.B):
                res['qT'][b] = resp.tile([P, c.HPC, c.NB], BF16, name=f"qT_{b}")
                res['kT'][b] = resp.tile([P, c.HPC, c.NB], BF16, name=f"kT_{b}")
                res['v'][b] = resp.tile([P, c.KCH, c.G], BF16, name=f"v_{b}")
            io['res'] = res

            def body():
                _phase_a(nc, tc, c, io, sc, res)
                _phase_bc(nc, tc, c, io, sc)

            if c.loop_iters > 1:
                with tc.For_i(0, c.loop_iters, 1):
                    body()
            else:
                body()

    nc.compile()
    return nc


# ---------------------------------------------------------------------------
# host side
# ---------------------------------------------------------------------------

def rope_tables(T, H, W, head_dim):
    """cos/sin tables [T*H*W, head_dim], mirroring reference._rope_freqs."""
    dim_t = head_dim - 4 * (head_dim // 6)
    dim_h = 2 * (head_dim // 6)
    dim_w = 2 * (head_dim // 6)
    base = 10000.0
    ft = 1.0 / base ** (np.arange(0, dim_t, 2)[: dim_t // 2].astype(np.float32) / dim_t)
    fh = 1.0 / base ** (np.arange(0, dim_h, 2)[: dim_h // 2].astype(np.float32) / dim_h)
    fw = 1.0 / base ** (np.arange(0, dim_w, 2)[: dim_w // 2].astype(np.float32) / dim_w)
    gt = np.arange(T, dtype=np.float32)
    gh = np.arange(H, dtype=np.float32)
    gw = np.arange(W, dtype=np.float32)
    Ft = np.repeat(gt[:, None] * ft[None, :], 2, axis=-1)
    Fh = np.repeat(gh[:, None] * fh[None, :], 2, axis=-1)
    Fw = np.repeat(gw[:, None] * fw[None, :], 2, axis=-1)
    Ft = np.broadcast_to(Ft[:, None, None, :], (T, H, W, Ft.shape[-1]))
    Fh = np.broadcast_to(Fh[None, :, None, :], (T, H, W, Fh.shape[-1]))
    Fw = np.broadcast_to(Fw[None, None, :, :], (T, H, W, Fw.shape[-1]))
    freqs = np.concatenate([Ft, Fh, Fw], axis=-1).reshape(T * H * W, head_dim)
    return np.cos(freqs).astype(np.float32), np.sin(freqs).astype(np.float32)


def signed_sin(sin, w_for_pairs):
    """sinW[2i] = -sin[2i]*w[2i+1]; sinW[2i+1] = sin[2i+1]*w[2i]."""
    out = np.empty_like(sin)
    out[:, 0::2] = -sin[:, 0::2] * w_for_pairs[None, 1::2]
    out[:, 1::2] = sin[:, 1::2] * w_for_pairs[None, 0::2]
    return out


def make_in_maps(x, qkv_w, qkv_b, q_norm_w, k_norm_w, proj_w,
                 cos, sin, NB, CIN, COUT, HPC, B, D=128, ncores=8):
    import ml_dtypes
    bf = ml_dtypes.bfloat16
    TOK = B * NB
    Hn = ncores * HPC
    C_heads = Hn * D
    xT = np.ascontiguousarray(x.reshape(TOK, CIN).T).astype(bf)
    cosq = (cos * q_norm_w[None, :]).astype(np.float32)
    cosk = (cos * k_norm_w[None, :]).astype(np.float32)
    sinq = signed_sin(sin, q_norm_w).astype(np.float32)
    sink = signed_sin(sin, k_norm_w).astype(np.float32)
    in_maps = []
    for cix in range(ncores):
        G = HPC * D
        r0 = cix * G
        w_local = np.concatenate([
            qkv_w[r0:r0 + G],
            qkv_w[C_heads + r0:C_heads + r0 + G],
            qkv_w[2 * C_heads + r0:2 * C_heads + r0 + G],
        ], axis=0)
        wT_local = np.ascontiguousarray(w_local.T).astype(bf)
        b_local = np.concatenate([
            qkv_b[r0:r0 + G],
            qkv_b[C_heads + r0:C_heads + r0 + G],
        ]).astype(np.float32)
        pwT_local = np.ascontiguousarray(proj_w[:, r0:r0 + G].T).astype(bf)
        in_maps.append({
            "xT": xT, "wT": wT_local, "bias": b_local,
            "cosq": cosq, "sinq": sinq, "cosk": cosk, "sink": sink,
            "pwT": pwT_local,
        })
    return in_maps


# ---------------------------------------------------------------------------
# harness entry point
# ---------------------------------------------------------------------------

_CACHE = {}

_B, _NB, _CIN, _COUT, _D, _NCORES, _HPC = 2, 2048, 3072, 3072, 128, 8, 3


def _get_program(rope_shared):
    key = ("prog", rope_shared)
    if key not in _CACHE:
        _CACHE[key] = build_program(NB=_NB, CIN=_CIN, COUT=_COUT, HPC=_HPC,
                                    B=_B, D=_D, rope_shared=rope_shared)
    return _CACHE[key]


def kernel(x, qkv_w, qkv_b, q_norm_w, k_norm_w, proj_w, proj_b,
           t_size, h_size, w_size):
    from concourse import bass_utils

    x = np.asarray(x, dtype=np.float32)
    qkv_w = np.asarray(qkv_w, dtype=np.float32)
    qkv_b = np.asarray(qkv_b, dtype=np.float32)
    q_norm_w = np.asarray(q_norm_w, dtype=np.float32)
    k_norm_w = np.asarray(k_norm_w, dtype=np.float32)
    proj_w = np.asarray(proj_w, dtype=np.float32)
    proj_b = np.asarray(proj_b, dtype=np.float32)

    cos, sin = rope_tables(int(t_size), int(h_size), int(w_size), _D)
    rope_shared = (np.array_equal(q_norm_w, k_norm_w))
    nc = _get_program(rope_shared)

    in_maps = make_in_maps(x, qkv_w, qkv_b, q_norm_w, k_norm_w, proj_w,
                           cos, sin, _NB, _CIN, _COUT, _HPC, _B, _D, _NCORES)
    res = bass_utils.run_bass_kernel_spmd(
        nc, in_maps, core_ids=list(range(_NCORES)), trace=False)
    part = np.zeros((_B * _NB, _COUT), np.float64)
    for r in res.results:
        part += r["out_part"].astype(np.float64)
    bv = qkv_b[2 * _CIN:].astype(np.float64)
    const = bv @ proj_w.astype(np.float64).T + proj_b.astype(np.float64)
    out = (part + const).reshape(_B, _NB, _COUT)
    return out.astype(np.float32)


# revision 31
# speedup vs baseline: 1.0713x; 1.0713x over previous
"""Trainium2 Bass kernel for the fused attention block
(QKV projection + RMSNorm + 3D RoPE + softmax attention + output projection),
tensor-parallel over heads across 8 NeuronCores.

Sharding: 3 heads per core. Each core computes its heads' QKV columns
(column-parallel), runs attention for (3 heads x 2 batches), and produces a
row-parallel partial of the output projection; the host sums the 8 partials
and adds proj_b (plus the folded v-bias term: attn rows sum to 1, so
attn@(v+bv) = attn@v + bv, and bv @ proj_w.T is a constant added on host).

Design vs the spill-based baseline:
 - bf16 data plane end-to-end (x, w, q/k/v, attention operands, proj weights,
   output partials); matmuls accumulate in fp32 PSUM, softmax/norm arithmetic
   in fp32.
 - q/k/v stay resident in SBUF (9.4 MB bf16); no DRAM spill round trip.
 - q/k transposed via the DMA XBAR (dma_start_transpose) instead of PE
   identity matmuls; PE does zero transpose work.
 - Phase B restructured per (batch, 512-query span): attention for all 3
   heads then immediately the output projection of that span, so the
   projection pipeline drains alongside attention instead of in a tail.
 - softmax denominators from matmuls against a [128,128] all-ones stationary:
   the key-sum lands replicated across every output partition, so the
   reciprocal applies directly with no broadcast of any kind.
 - v-bias folded into the host-side constant (attn rows sum to 1, so
   attn@(v+bv) = attn@v + bv and bv@proj_w.T is added once on host).
"""
import sys
sys.path.insert(0, '/opt/trn_rl_repo')

import numpy as np
import concourse.bass as bass
import concourse.mybir as mybir
import concourse.tile as tile
from concourse import bacc
from concourse.bass import ts, ds

F32 = mybir.dt.float32
F32R = mybir.dt.float32r
BF16 = mybir.dt.bfloat16
AF = mybir.ActivationFunctionType
P = 128


class Cfg:
    def __init__(self, NB=2048, CIN=3072, COUT=3072, HPC=3, B=2, D=128,
                 eps=1e-6, loop_iters=1, rope_shared=True, newton_iters=3,
                 xt_bufs=3, stps_bufs=2, outps_bufs=1, pexp_bufs=10,
                 pcps_bufs=2, SPAN=512, pe_transpose=False, dma_split=True,
                 fp8_den=False):
        self.__dict__.update(locals())
        del self.__dict__['self']
        self.TOK = B * NB
        self.KC = CIN // P           # contraction chunks for qkv matmul
        self.G = HPC * D             # per-core head channels (per q/k/v)
        self.TT = self.TOK // P      # token tiles total
        self.TPB = NB // P           # token tiles per batch
        self.NSPAN = NB // SPAN      # query spans per batch
        self.KCH = NB // P           # key chunks per batch
        self.OC = 512
        self.NOC = COUT // self.OC
        self.TPS = SPAN // P         # token tiles per span
        assert NB % SPAN == 0 and CIN % P == 0 and COUT % self.OC == 0


def _phase_a(nc, tc, c, io, sc, res):
    """QKV matmul + bias + rms-norm + rope; q/k into SBUF transposed via the
    DMA XBAR, v into SBUF in [tok, d] layout. All bf16."""
    D, G, KC, TT, TPB, HPC = c.D, c.G, c.KC, c.TT, c.TPB, c.HPC
    MUL, ADD = mybir.AluOpType.mult, mybir.AluOpType.add
    qdma = nc.scalar if c.dma_split else nc.sync
    with tc.tile_pool(name="paconst", bufs=1) as paconst, \
         tc.tile_pool(name="pa", bufs=2) as pa, \
         tc.tile_pool(name="paps", bufs=2, space="PSUM") as paps, \
         tc.tile_pool(name="tpps", bufs=2, space="PSUM") as tpps:
        x2 = io['x2']
        wT3 = io['wT'].rearrange("(kc p) g -> p kc g", p=P)
        # x tiles stream on the SP queue; weights/tables on the ACT queue so
        # the first matmuls aren't stuck behind the 7MB weight load
        xt0 = pa.tile([P, KC, P], BF16, tag="xt", name="xt_0", bufs=c.xt_bufs)
        nc.sync.dma_start(xt0, x2[0])
        w_sb = paconst.tile([P, KC, 3 * G], BF16)
        for kc in range(KC):
            qdma.dma_start(w_sb[:, ds(kc, 1)], wT3[:, ds(kc, 1)])
        if c.pe_transpose:
            from concourse.masks import make_identity
            ident = paconst.tile([P, P], BF16, name="ident")
            make_identity(nc, ident)
        bias_sb = paconst.tile([P, 2 * G], F32)
        nc.gpsimd.dma_start(bias_sb, io['bias'].partition_broadcast(P))
        if c.rope_shared:
            cos_t = paconst.tile([P, TPB, D], F32, name="cos_t")
            sin_t = paconst.tile([P, TPB, D], F32, name="sin_t")
            qdma.dma_start(cos_t, io['cosq'].rearrange("(tt p) d -> p tt d", p=P))
            qdma.dma_start(sin_t, io['sinq'].rearrange("(tt p) d -> p tt d", p=P))
            cos_sb = {"q": cos_t, "k": cos_t}
            sin_sb = {"q": sin_t, "k": sin_t}
        else:
            cos_sb, sin_sb = {}, {}
            for nm in ("q", "k"):
                ct = paconst.tile([P, TPB, D], F32, tag=f"cos_{nm}", name=f"cos_{nm}")
                st2 = paconst.tile([P, TPB, D], F32, tag=f"sin_{nm}", name=f"sin_{nm}")
                qdma.dma_start(ct, io[f'cos{nm}'].rearrange("(tt p) d -> p tt d", p=P))
                qdma.dma_start(st2, io[f'sin{nm}'].rearrange("(tt p) d -> p tt d", p=P))
                cos_sb[nm], sin_sb[nm] = ct, st2
        for t in range(TT):
            b, tb = t // TPB, t % TPB
            if t == 0:
                xt = xt0
            else:
                xt = pa.tile([P, KC, P], BF16, tag="xt", name=f"xt_{t}",
                             bufs=c.xt_bufs)
                nc.sync.dma_start(xt, x2[t])
            ps = {}
            for s, name in enumerate(("q", "k", "v")):
                ps[name] = paps.tile([P, G], F32, tag=f"ps_{name}",
                                     name=f"ps_{name}_{t}", bufs=2)
            for kc in range(KC):
                for s, name in enumerate(("q", "k", "v")):
                    nc.tensor.matmul(ps[name], xt[:, kc],
                                     w_sb[:, kc, ds(s * G, G)],
                                     start=(kc == 0), stop=(kc == KC - 1))
            # v: PSUM -> resident SBUF bf16 (bias folded into host proj_b);
            # Pool can't read PSUM, so this goes on ACT
            nc.scalar.copy(res['v'][b][:, ds(tb, 1)],
                           ps["v"].rearrange("p (o g) -> p o g", o=1))
            for si, name in enumerate(("q", "k")):
                raw = pa.tile([P, G], F32, tag="raw")
                nc.vector.tensor_add(raw, ps[name], bias_sb[:, ds(si * G, G)])
                raw3 = raw.rearrange("p (h d) -> p h d", d=D)
                # per-head sum of squares on ACT
                ssum = pa.tile([P, HPC], F32, tag="ssum")
                sqscr = pa.tile([P, D], F32, tag="sqscr")
                for h in range(HPC):
                    nc.scalar.activation(sqscr, raw3[:, h], AF.Square,
                                         accum_out=ssum[:, ds(h, 1)])
                # m = ssum/D + eps; rstd = rsqrt(m) via Newton on DVE
                m_t = pa.tile([P, HPC], F32, tag="m_t")
                nc.vector.tensor_scalar(m_t, ssum, 1.0 / D, c.eps, MUL, ADD)
                rstd = pa.tile([P, HPC], F32, tag="rstd")
                nc.vector.tensor_scalar(rstd, m_t, -0.5, 1.5, MUL, ADD)
                nt1 = pa.tile([P, HPC], F32, tag="nt1")
                for _ in range(c.newton_iters):
                    nc.vector.tensor_mul(nt1, rstd, rstd)
                    nc.vector.tensor_mul(nt1, nt1, m_t)
                    nc.vector.tensor_scalar(nt1, nt1, -0.5, 1.5, MUL, ADD)
                    nc.vector.tensor_mul(rstd, rstd, nt1)
                # pair swap on Pool (sw[2i]=raw[2i+1], sw[2i+1]=raw[2i])
                sw = pa.tile([P, HPC, D], F32, tag="sw")
                raw2 = raw.rearrange("p (a two) -> p a two", two=2)
                sw2 = sw.rearrange("p h (a two) -> p (h a) two", two=2)
                nc.gpsimd.tensor_copy(sw2[:, :, ds(0, 1)], raw2[:, :, ds(1, 1)])
                nc.gpsimd.tensor_copy(sw2[:, :, ds(1, 1)], raw2[:, :, ds(0, 1)])
                # rope fused with rstd apply: ro = (raw*rstd)*cosW + (sw*rstd)*sinW
                ro = pa.tile([P, HPC, D], BF16, tag=f"ro_{name}")
                rtmp = pa.tile([P, HPC, D], F32, tag="rtmp")
                rtmp2 = pa.tile([P, HPC, D], F32, tag="rtmp2")
                for h in range(HPC):
                    nc.vector.scalar_tensor_tensor(
                        rtmp[:, h], sw[:, h], rstd[:, ds(h, 1)],
                        sin_sb[name][:, tb], MUL, MUL)
                    nc.vector.scalar_tensor_tensor(
                        rtmp2[:, h], raw3[:, h], rstd[:, ds(h, 1)],
                        cos_sb[name][:, tb], MUL, MUL)
                    nc.vector.tensor_add(ro[:, h], rtmp2[:, h], rtmp[:, h])
                dst = res['qT'] if name == "q" else res['kT']
                for h in range(HPC):
                    if c.pe_transpose:
                        tp = tpps.tile([P, P], BF16, tag="tp")
                        nc.tensor.matmul(tp, ro[:, h], ident, is_transpose=True)
                        nc.vector.tensor_copy(dst[b][:, h, ts(tb, P)], tp)
                    else:
                        nc.sync.dma_start_transpose(dst[b][:, h, ts(tb, P)], ro[:, h])


def _phase_bc(nc, tc, c, io, sc):
    """Per (batch, query-span): attention for all heads, then that span's
    output projection."""
    D, HPC, B = c.D, c.HPC, c.B
    NB, SPAN, NSPAN, KCH = c.NB, c.SPAN, c.NSPAN, c.KCH
    NOC, OC, COUT, TPS = c.NOC, c.OC, c.COUT, c.TPS
    res = io['res']
    scale = float(D) ** -0.5
    with tc.tile_pool(name="pb", bufs=2) as pb, \
         tc.tile_pool(name="pbc", bufs=1) as pbc, \
         tc.tile_pool(name="pexpp", bufs=c.pexp_bufs) as pexpp, \
         tc.tile_pool(name="stps", bufs=c.stps_bufs, space="PSUM") as stps, \
         tc.tile_pool(name="outps", bufs=c.outps_bufs, space="PSUM") as outps, \
         tc.tile_pool(name="denps", bufs=1, space="PSUM") as denps, \
         tc.tile_pool(name="pcps", bufs=c.pcps_bufs, space="PSUM") as pcps:
        pw_sb = pbc.tile([P, HPC, COUT], BF16, bufs=1)
        pwT3 = io['pwT'].rearrange("(h p) o -> p h o", p=P)
        qdma = nc.scalar if c.dma_split else nc.sync
        for h in range(HPC):
            qdma.dma_start(pw_sb[:, ds(h, 1)], pwT3[:, ds(h, 1)])
        for b in range(B):
            for s in range(NSPAN):
                at = {}
                for h in range(HPC):
                    qspan = res['qT'][b][:, h, ds(s * SPAN, SPAN)]
                    pexps = []
                    for pr in range(KCH // 2):
                        stp = stps.tile([P, 2 * SPAN], F32, tag="stp")
                        for j in (0, 1):
                            kc = 2 * pr + j
                            nc.tensor.matmul(stp[:, ds(j * SPAN, SPAN)],
                                             res['kT'][b][:, h, ts(kc, P)],
                                             qspan, start=True, stop=True)
                        pexp = pexpp.tile([P, 2 * SPAN], BF16, tag="pexp")
                        nc.scalar.activation(pexp, stp, AF.Exp,
                                             bias=sc['gate'], scale=scale)
                        pexps.append(pexp)
                        if pr == 0:
                            outp = outps.tile([P, SPAN], F32, tag="outp")
                        for j in (0, 1):
                            kc = 2 * pr + j
                            nc.tensor.matmul(outp, res['v'][b][:, kc, ds(h * D, D)],
                                             pexp[:, ds(j * SPAN, SPAN)],
                                             start=(kc == 0), stop=(kc == KCH - 1))
                    # den with a [128,128] all-ones stationary: every output
                    # partition gets the key-sum, so no broadcast is needed
                    denp = denps.tile([P, SPAN], F32, tag="denp")
                    for pr in range(KCH // 2):
                        for j in (0, 1):
                            kc = 2 * pr + j
                            nc.tensor.matmul(denp, sc['ones_sq'],
                                             pexps[pr][:, ds(j * SPAN, SPAN)],
                                             start=(kc == 0), stop=(kc == KCH - 1))
                    drep = pb.tile([P, SPAN], F32, tag="drep")
                    nc.vector.reciprocal(drep, denp)
                    ath = pb.tile([P, SPAN], BF16, tag=f"at_{h}")
                    nc.vector.tensor_mul(ath, outp, drep)
                    at[h] = ath
                # ---- projection for this span ----
                for tb in range(TPS):
                    t = b * (NB // P) + s * TPS + tb
                    for o in range(NOC):
                        op = pcps.tile([P, OC], F32, tag="op", name=f"op_{t}_{o}")
                        for h in range(HPC):
                            nc.tensor.matmul(op, at[h][:, ts(tb, P)],
                                             pw_sb[:, h, ds(o * OC, OC)],
                                             start=(h == 0), stop=(h == HPC - 1))
                        ost = pbc.tile([P, OC], BF16, tag="ost",
                                       name=f"ost_{t}_{o}", bufs=6)
                        # Pool can't read PSUM: rotate DVE-heavy (bf16 out
                        # runs 2x on DVE) with ACT taking every third
                        if (tb * NOC + o) % 3 == 2:
                            nc.scalar.copy(ost, op)
                        else:
                            nc.vector.tensor_copy(ost, op)
                        nc.sync.dma_start(io['out_part'][ts(t, P), ds(o * OC, OC)],
                                          ost)


def build_program(**kw):
    c = Cfg(**kw)
    nc = bacc.Bacc("TRN2", target_bir_lowering=False, debug=False,
                   enable_asserts=False, num_devices=8)

    io = {}
    io['x2'] = nc.dram_tensor("x2", [c.TT, P, c.KC, P], BF16,
                              kind="ExternalInput").ap()
    io['wT'] = nc.dram_tensor("wT", [c.CIN, 3 * c.G], BF16, kind="ExternalInput").ap()
    io['bias'] = nc.dram_tensor("bias", [2 * c.G], F32, kind="ExternalInput").ap()
    for nm in ("q", "k"):
        io[f'cos{nm}'] = nc.dram_tensor(f"cos{nm}", [c.NB, c.D], F32,
                                        kind="ExternalInput").ap()
        io[f'sin{nm}'] = nc.dram_tensor(f"sin{nm}", [c.NB, c.D], F32,
                                        kind="ExternalInput").ap()
    io['pwT'] = nc.dram_tensor("pwT", [c.G, c.COUT], BF16, kind="ExternalInput").ap()
    io['out_part'] = nc.dram_tensor("out_part", [c.TOK, c.COUT], BF16,
                                    kind="ExternalOutput").ap()

    with tile.TileContext(nc) as tc:
        with tc.tile_pool(name="const", bufs=1) as constp, \
             tc.tile_pool(name="resp", bufs=1) as resp:
            sc = {}
            ones_sq_f = constp.tile([P, P], F32)
            nc.vector.memset(ones_sq_f, 1.0)
            ones_sq = constp.tile([P, P], BF16)
            nc.vector.tensor_copy(ones_sq, ones_sq_f)
            gate = constp.tile([P, 1], F32)
            nc.vector.memset(gate, 0.0)
            sc.update(ones_sq=ones_sq, gate=gate)

            res = {'qT': {}, 'kT': {}, 'v': {}}
            for b in range(c.B):
                res['qT'][b] = resp.tile([P, c.HPC, c.NB], BF16, name=f"qT_{b}")
                res['kT'][b] = resp.tile([P, c.HPC, c.NB], BF16, name=f"kT_{b}")
                res['v'][b] = resp.tile([P, c.KCH, c.G], BF16, name=f"v_{b}")
            io['res'] = res

            def body():
                _phase_a(nc, tc, c, io, sc, res)
                _phase_bc(nc, tc, c, io, sc)

            if c.loop_iters > 1:
                with tc.For_i(0, c.loop_iters, 1):
                    body()
            else:
                body()

    nc.compile()
    return nc


# ---------------------------------------------------------------------------
# host side
# ---------------------------------------------------------------------------

def rope_tables(T, H, W, head_dim):
    """cos/sin tables [T*H*W, head_dim], mirroring reference._rope_freqs."""
    dim_t = head_dim - 4 * (head_dim // 6)
    dim_h = 2 * (head_dim // 6)
    dim_w = 2 * (head_dim // 6)
    base = 10000.0
    ft = 1.0 / base ** (np.arange(0, dim_t, 2)[: dim_t // 2].astype(np.float32) / dim_t)
    fh = 1.0 / base ** (np.arange(0, dim_h, 2)[: dim_h // 2].astype(np.float32) / dim_h)
    fw = 1.0 / base ** (np.arange(0, dim_w, 2)[: dim_w // 2].astype(np.float32) / dim_w)
    gt = np.arange(T, dtype=np.float32)
    gh = np.arange(H, dtype=np.float32)
    gw = np.arange(W, dtype=np.float32)
    Ft = np.repeat(gt[:, None] * ft[None, :], 2, axis=-1)
    Fh = np.repeat(gh[:, None] * fh[None, :], 2, axis=-1)
    Fw = np.repeat(gw[:, None] * fw[None, :], 2, axis=-1)
    Ft = np.broadcast_to(Ft[:, None, None, :], (T, H, W, Ft.shape[-1]))
    Fh = np.broadcast_to(Fh[None, :, None, :], (T, H, W, Fh.shape[-1]))
    Fw = np.broadcast_to(Fw[None, None, :, :], (T, H, W, Fw.shape[-1]))
    freqs = np.concatenate([Ft, Fh, Fw], axis=-1).reshape(T * H * W, head_dim)
    return np.cos(freqs).astype(np.float32), np.sin(freqs).astype(np.float32)


def signed_sin(sin, w_for_pairs):
    """sinW[2i] = -sin[2i]*w[2i+1]; sinW[2i+1] = sin[2i+1]*w[2i]."""
    out = np.empty_like(sin)
    out[:, 0::2] = -sin[:, 0::2] * w_for_pairs[None, 1::2]
    out[:, 1::2] = sin[:, 1::2] * w_for_pairs[None, 0::2]
    return out


def make_in_maps(x, qkv_w, qkv_b, q_norm_w, k_norm_w, proj_w,
                 cos, sin, NB, CIN, COUT, HPC, B, D=128, ncores=8):
    import ml_dtypes
    bf = ml_dtypes.bfloat16
    TOK = B * NB
    Hn = ncores * HPC
    C_heads = Hn * D
    xT = x.reshape(TOK, CIN).T.astype(bf)
    KCn, TTn = CIN // D, TOK // D
    x2 = np.ascontiguousarray(
        xT.reshape(KCn, D, TTn, D).transpose(2, 1, 0, 3))
    cosq = (cos * q_norm_w[None, :]).astype(np.float32)
    cosk = (cos * k_norm_w[None, :]).astype(np.float32)
    sinq = signed_sin(sin, q_norm_w).astype(np.float32)
    sink = signed_sin(sin, k_norm_w).astype(np.float32)
    in_maps = []
    for cix in range(ncores):
        G = HPC * D
        r0 = cix * G
        w_local = np.concatenate([
            qkv_w[r0:r0 + G],
            qkv_w[C_heads + r0:C_heads + r0 + G],
            qkv_w[2 * C_heads + r0:2 * C_heads + r0 + G],
        ], axis=0)
        wT_local = np.ascontiguousarray(w_local.T).astype(bf)
        b_local = np.concatenate([
            qkv_b[r0:r0 + G],
            qkv_b[C_heads + r0:C_heads + r0 + G],
        ]).astype(np.float32)
        pwT_local = np.ascontiguousarray(proj_w[:, r0:r0 + G].T).astype(bf)
        in_maps.append({
            "x2": x2, "wT": wT_local, "bias": b_local,
            "cosq": cosq, "sinq": sinq, "cosk": cosk, "sink": sink,
            "pwT": pwT_local,
        })
    return in_maps


# ---------------------------------------------------------------------------
# harness entry point
# ---------------------------------------------------------------------------

_CACHE = {}

_B, _NB, _CIN, _COUT, _D, _NCORES, _HPC = 2, 2048, 3072, 3072, 128, 8, 3


def _get_program(rope_shared):
    key = ("prog", rope_shared)
    if key not in _CACHE:
        _CACHE[key] = build_program(NB=_NB, CIN=_CIN, COUT=_COUT, HPC=_HPC,
                                    B=_B, D=_D, rope_shared=rope_shared)
    return _CACHE[key]


def kernel(x, qkv_w, qkv_b, q_norm_w, k_norm_w, proj_w, proj_b,
           t_size, h_size, w_size):
    from concourse import bass_utils

    x = np.asarray(x, dtype=np.float32)
    qkv_w = np.asarray(qkv_w, dtype=np.float32)
    qkv_b = np.asarray(qkv_b, dtype=np.float32)
    q_norm_w = np.asarray(q_norm_w, dtype=np.float32)
    k_norm_w = np.asarray(k_norm_w, dtype=np.float32)
    proj_w = np.asarray(proj_w, dtype=np.float32)
    proj_b = np.asarray(proj_b, dtype=np.float32)

    cos, sin = rope_tables(int(t_size), int(h_size), int(w_size), _D)
    rope_shared = (np.array_equal(q_norm_w, k_norm_w))
    nc = _get_program(rope_shared)

    in_maps = make_in_maps(x, qkv_w, qkv_b, q_norm_w, k_norm_w, proj_w,
                           cos, sin, _NB, _CIN, _COUT, _HPC, _B, _D, _NCORES)
    res = bass_utils.run_bass_kernel_spmd(
        nc, in_maps, core_ids=list(range(_NCORES)), trace=False)
    part = np.zeros((_B * _NB, _COUT), np.float64)
    for r in res.results:
        part += r["out_part"].astype(np.float64)
    bv = qkv_b[2 * _CIN:].astype(np.float64)
    const = bv @ proj_w.astype(np.float64).T + proj_b.astype(np.float64)
    out = (part + const).reshape(_B, _NB, _COUT)
    return out.astype(np.float32)


# revision 32
# speedup vs baseline: 1.0930x; 1.0202x over previous
"""Trainium2 Bass kernel for the fused attention block
(QKV projection + RMSNorm + 3D RoPE + softmax attention + output projection),
tensor-parallel over heads across 8 NeuronCores.

Sharding: 3 heads per core. Each core computes its heads' QKV columns
(column-parallel), runs attention for (3 heads x 2 batches), and produces a
row-parallel partial of the output projection; the host sums the 8 partials
and adds proj_b (plus the folded v-bias term: attn rows sum to 1, so
attn@(v+bv) = attn@v + bv, and bv @ proj_w.T is a constant added on host).

Design vs the spill-based baseline:
 - bf16 data plane end-to-end (x, w, q/k/v, attention operands, proj weights,
   output partials); matmuls accumulate in fp32 PSUM, softmax/norm arithmetic
   in fp32.
 - q/k/v stay resident in SBUF (9.4 MB bf16); no DRAM spill round trip.
 - q/k transposed via the DMA XBAR (dma_start_transpose) instead of PE
   identity matmuls; PE does zero transpose work.
 - Phase B restructured per (batch, 512-query span): attention for all 3
   heads then immediately the output projection of that span, so the
   projection pipeline drains alongside attention instead of in a tail.
 - softmax denominators from matmuls against a [128,128] all-ones stationary:
   the key-sum lands replicated across every output partition, so the
   reciprocal applies directly with no broadcast of any kind.
 - v-bias folded into the host-side constant (attn rows sum to 1, so
   attn@(v+bv) = attn@v + bv and bv@proj_w.T is added once on host).
"""
import sys
sys.path.insert(0, '/opt/trn_rl_repo')

import numpy as np
import concourse.bass as bass
import concourse.mybir as mybir
import concourse.tile as tile
from concourse import bacc
from concourse.bass import ts, ds

F32 = mybir.dt.float32
F32R = mybir.dt.float32r
BF16 = mybir.dt.bfloat16
AF = mybir.ActivationFunctionType
P = 128


class Cfg:
    def __init__(self, NB=2048, CIN=3072, COUT=3072, HPC=3, B=2, D=128,
                 eps=1e-6, loop_iters=1, rope_shared=True, newton_iters=3,
                 xt_bufs=4, stps_bufs=2, outps_bufs=1, pexp_bufs=10,
                 pcps_bufs=2, SPAN=512, pe_transpose=False, dma_split=True,
                 fp8_den=False):
        self.__dict__.update(locals())
        del self.__dict__['self']
        self.TOK = B * NB
        self.KC = CIN // P           # contraction chunks for qkv matmul
        self.G = HPC * D             # per-core head channels (per q/k/v)
        self.TT = self.TOK // P      # token tiles total
        self.TPB = NB // P           # token tiles per batch
        self.NSPAN = NB // SPAN      # query spans per batch
        self.KCH = NB // P           # key chunks per batch
        self.OC = 512
        self.NOC = COUT // self.OC
        self.TPS = SPAN // P         # token tiles per span
        assert NB % SPAN == 0 and CIN % P == 0 and COUT % self.OC == 0


def _phase_a(nc, tc, c, io, sc, res):
    """QKV matmul + bias + rms-norm + rope; q/k into SBUF transposed via the
    DMA XBAR, v into SBUF in [tok, d] layout. All bf16."""
    D, G, KC, TT, TPB, HPC = c.D, c.G, c.KC, c.TT, c.TPB, c.HPC
    MUL, ADD = mybir.AluOpType.mult, mybir.AluOpType.add
    qdma = nc.scalar if c.dma_split else nc.sync
    with tc.tile_pool(name="paconst", bufs=1) as paconst, \
         tc.tile_pool(name="pa", bufs=2) as pa, \
         tc.tile_pool(name="paps", bufs=2, space="PSUM") as paps, \
         tc.tile_pool(name="tpps", bufs=2, space="PSUM") as tpps:
        x2 = io['x2']
        wT3 = io['wT'].rearrange("(kc p) g -> p kc g", p=P)
        # x tiles stream on the SP queue; weights/tables on the ACT queue so
        # the first matmuls aren't stuck behind the 7MB weight load
        xt0 = pa.tile([P, KC, P], BF16, tag="xt", name="xt_0", bufs=c.xt_bufs)
        nc.sync.dma_start(xt0, x2[0])
        w_sb = paconst.tile([P, KC, 3 * G], BF16)
        for kc in range(KC):
            qdma.dma_start(w_sb[:, ds(kc, 1)], wT3[:, ds(kc, 1)])
        if c.pe_transpose:
            from concourse.masks import make_identity
            ident = paconst.tile([P, P], BF16, name="ident")
            make_identity(nc, ident)
        bias_sb = paconst.tile([P, 2 * G], F32)
        nc.gpsimd.dma_start(bias_sb, io['bias'].partition_broadcast(P))
        if c.rope_shared:
            cos_t = paconst.tile([P, TPB, D], F32, name="cos_t")
            sin_t = paconst.tile([P, TPB, D], F32, name="sin_t")
            qdma.dma_start(cos_t, io['cosq'])
            qdma.dma_start(sin_t, io['sinq'])
            cos_sb = {"q": cos_t, "k": cos_t}
            sin_sb = {"q": sin_t, "k": sin_t}
        else:
            cos_sb, sin_sb = {}, {}
            for nm in ("q", "k"):
                ct = paconst.tile([P, TPB, D], F32, tag=f"cos_{nm}", name=f"cos_{nm}")
                st2 = paconst.tile([P, TPB, D], F32, tag=f"sin_{nm}", name=f"sin_{nm}")
                qdma.dma_start(ct, io[f'cos{nm}'])
                qdma.dma_start(st2, io[f'sin{nm}'])
                cos_sb[nm], sin_sb[nm] = ct, st2
        for t in range(TT):
            b, tb = t // TPB, t % TPB
            if t == 0:
                xt = xt0
            else:
                xt = pa.tile([P, KC, P], BF16, tag="xt", name=f"xt_{t}",
                             bufs=c.xt_bufs)
                nc.sync.dma_start(xt, x2[t])
            ps = {}
            for s, name in enumerate(("q", "k", "v")):
                ps[name] = paps.tile([P, G], F32, tag=f"ps_{name}",
                                     name=f"ps_{name}_{t}", bufs=2)
            for kc in range(KC):
                for s, name in enumerate(("q", "k", "v")):
                    nc.tensor.matmul(ps[name], xt[:, kc],
                                     w_sb[:, kc, ds(s * G, G)],
                                     start=(kc == 0), stop=(kc == KC - 1))
            # v: PSUM -> resident SBUF bf16 (bias folded into host proj_b);
            # Pool can't read PSUM, so this goes on ACT
            nc.scalar.copy(res['v'][b][:, ds(tb, 1)],
                           ps["v"].rearrange("p (o g) -> p o g", o=1))
            for si, name in enumerate(("q", "k")):
                raw = pa.tile([P, G], F32, tag="raw")
                nc.vector.tensor_add(raw, ps[name], bias_sb[:, ds(si * G, G)])
                raw3 = raw.rearrange("p (h d) -> p h d", d=D)
                # per-head sum of squares on ACT
                ssum = pa.tile([P, HPC], F32, tag="ssum")
                sqscr = pa.tile([P, D], F32, tag="sqscr")
                for h in range(HPC):
                    nc.scalar.activation(sqscr, raw3[:, h], AF.Square,
                                         accum_out=ssum[:, ds(h, 1)])
                # m = ssum/D + eps; rstd = rsqrt(m) via Newton on DVE
                m_t = pa.tile([P, HPC], F32, tag="m_t")
                nc.vector.tensor_scalar(m_t, ssum, 1.0 / D, c.eps, MUL, ADD)
                rstd = pa.tile([P, HPC], F32, tag="rstd")
                nc.vector.tensor_scalar(rstd, m_t, -0.5, 1.5, MUL, ADD)
                nt1 = pa.tile([P, HPC], F32, tag="nt1")
                for _ in range(c.newton_iters):
                    nc.vector.tensor_mul(nt1, rstd, rstd)
                    nc.vector.tensor_mul(nt1, nt1, m_t)
                    nc.vector.tensor_scalar(nt1, nt1, -0.5, 1.5, MUL, ADD)
                    nc.vector.tensor_mul(rstd, rstd, nt1)
                # pair swap on Pool (sw[2i]=raw[2i+1], sw[2i+1]=raw[2i])
                sw = pa.tile([P, HPC, D], F32, tag="sw")
                raw2 = raw.rearrange("p (a two) -> p a two", two=2)
                sw2 = sw.rearrange("p h (a two) -> p (h a) two", two=2)
                nc.gpsimd.tensor_copy(sw2[:, :, ds(0, 1)], raw2[:, :, ds(1, 1)])
                nc.gpsimd.tensor_copy(sw2[:, :, ds(1, 1)], raw2[:, :, ds(0, 1)])
                # rope fused with rstd apply: ro = (raw*rstd)*cosW + (sw*rstd)*sinW
                ro = pa.tile([P, HPC, D], BF16, tag=f"ro_{name}")
                rtmp = pa.tile([P, HPC, D], F32, tag="rtmp")
                rtmp2 = pa.tile([P, HPC, D], F32, tag="rtmp2")
                for h in range(HPC):
                    nc.vector.scalar_tensor_tensor(
                        rtmp[:, h], sw[:, h], rstd[:, ds(h, 1)],
                        sin_sb[name][:, tb], MUL, MUL)
                    nc.vector.scalar_tensor_tensor(
                        rtmp2[:, h], raw3[:, h], rstd[:, ds(h, 1)],
                        cos_sb[name][:, tb], MUL, MUL)
                    nc.vector.tensor_add(ro[:, h], rtmp2[:, h], rtmp[:, h])
                dst = res['qT'] if name == "q" else res['kT']
                for h in range(HPC):
                    if c.pe_transpose:
                        tp = tpps.tile([P, P], BF16, tag="tp")
                        nc.tensor.matmul(tp, ro[:, h], ident, is_transpose=True)
                        nc.vector.tensor_copy(dst[b][:, h, ts(tb, P)], tp)
                    else:
                        nc.sync.dma_start_transpose(dst[b][:, h, ts(tb, P)], ro[:, h])


def _phase_bc(nc, tc, c, io, sc):
    """Per (batch, query-span): attention for all heads, then that span's
    output projection."""
    D, HPC, B = c.D, c.HPC, c.B
    NB, SPAN, NSPAN, KCH = c.NB, c.SPAN, c.NSPAN, c.KCH
    NOC, OC, COUT, TPS = c.NOC, c.OC, c.COUT, c.TPS
    res = io['res']
    scale = float(D) ** -0.5
    with tc.tile_pool(name="pb", bufs=2) as pb, \
         tc.tile_pool(name="pbc", bufs=1) as pbc, \
         tc.tile_pool(name="pexpp", bufs=c.pexp_bufs) as pexpp, \
         tc.tile_pool(name="stps", bufs=c.stps_bufs, space="PSUM") as stps, \
         tc.tile_pool(name="outps", bufs=c.outps_bufs, space="PSUM") as outps, \
         tc.tile_pool(name="denps", bufs=1, space="PSUM") as denps, \
         tc.tile_pool(name="pcps", bufs=c.pcps_bufs, space="PSUM") as pcps:
        pw_sb = pbc.tile([P, HPC, COUT], BF16, bufs=1)
        pwT3 = io['pwT'].rearrange("(h p) o -> p h o", p=P)
        qdma = nc.scalar if c.dma_split else nc.sync
        for h in range(HPC):
            qdma.dma_start(pw_sb[:, ds(h, 1)], pwT3[:, ds(h, 1)])
        for b in range(B):
            for s in range(NSPAN):
                at = {}
                for h in range(HPC):
                    qspan = res['qT'][b][:, h, ds(s * SPAN, SPAN)]
                    pexps = []
                    for pr in range(KCH // 2):
                        stp = stps.tile([P, 2 * SPAN], F32, tag="stp")
                        for j in (0, 1):
                            kc = 2 * pr + j
                            nc.tensor.matmul(stp[:, ds(j * SPAN, SPAN)],
                                             res['kT'][b][:, h, ts(kc, P)],
                                             qspan, start=True, stop=True)
                        pexp = pexpp.tile([P, 2 * SPAN], BF16, tag="pexp")
                        nc.scalar.activation(pexp, stp, AF.Exp,
                                             bias=sc['gate'], scale=scale)
                        pexps.append(pexp)
                        if pr == 0:
                            outp = outps.tile([P, SPAN], F32, tag="outp")
                        for j in (0, 1):
                            kc = 2 * pr + j
                            nc.tensor.matmul(outp, res['v'][b][:, kc, ds(h * D, D)],
                                             pexp[:, ds(j * SPAN, SPAN)],
                                             start=(kc == 0), stop=(kc == KCH - 1))
                    # den with a [128,128] all-ones stationary: every output
                    # partition gets the key-sum, so no broadcast is needed
                    denp = denps.tile([P, SPAN], F32, tag="denp")
                    for pr in range(KCH // 2):
                        for j in (0, 1):
                            kc = 2 * pr + j
                            nc.tensor.matmul(denp, sc['ones_sq'],
                                             pexps[pr][:, ds(j * SPAN, SPAN)],
                                             start=(kc == 0), stop=(kc == KCH - 1))
                    drep = pb.tile([P, SPAN], F32, tag="drep")
                    nc.vector.reciprocal(drep, denp)
                    ath = pb.tile([P, SPAN], BF16, tag=f"at_{h}")
                    nc.vector.tensor_mul(ath, outp, drep)
                    at[h] = ath
                # ---- projection for this span ----
                for tb in range(TPS):
                    t = b * (NB // P) + s * TPS + tb
                    for o in range(NOC):
                        op = pcps.tile([P, OC], F32, tag="op", name=f"op_{t}_{o}")
                        for h in range(HPC):
                            nc.tensor.matmul(op, at[h][:, ts(tb, P)],
                                             pw_sb[:, h, ds(o * OC, OC)],
                                             start=(h == 0), stop=(h == HPC - 1))
                        ost = pbc.tile([P, OC], BF16, tag="ost",
                                       name=f"ost_{t}_{o}", bufs=6)
                        # Pool can't read PSUM: rotate DVE-heavy (bf16 out
                        # runs 2x on DVE) with ACT taking every third
                        if (tb * NOC + o) % 3 == 2:
                            nc.scalar.copy(ost, op)
                        else:
                            nc.vector.tensor_copy(ost, op)
                        nc.sync.dma_start(io['out_part'][ts(t, P), ds(o * OC, OC)],
                                          ost)


def build_program(**kw):
    c = Cfg(**kw)
    nc = bacc.Bacc("TRN2", target_bir_lowering=False, debug=False,
                   enable_asserts=False, num_devices=8)

    io = {}
    io['x2'] = nc.dram_tensor("x2", [c.TT, P, c.KC, P], BF16,
                              kind="ExternalInput").ap()
    io['wT'] = nc.dram_tensor("wT", [c.CIN, 3 * c.G], BF16, kind="ExternalInput").ap()
    io['bias'] = nc.dram_tensor("bias", [2 * c.G], F32, kind="ExternalInput").ap()
    for nm in ("q", "k"):
        io[f'cos{nm}'] = nc.dram_tensor(f"cos{nm}", [P, c.TPB, c.D], F32,
                                        kind="ExternalInput").ap()
        io[f'sin{nm}'] = nc.dram_tensor(f"sin{nm}", [P, c.TPB, c.D], F32,
                                        kind="ExternalInput").ap()
    io['pwT'] = nc.dram_tensor("pwT", [c.G, c.COUT], BF16, kind="ExternalInput").ap()
    io['out_part'] = nc.dram_tensor("out_part", [c.TOK, c.COUT], BF16,
                                    kind="ExternalOutput").ap()

    with tile.TileContext(nc) as tc:
        with tc.tile_pool(name="const", bufs=1) as constp, \
             tc.tile_pool(name="resp", bufs=1) as resp:
            sc = {}
            ones_sq_f = constp.tile([P, P], F32)
            nc.vector.memset(ones_sq_f, 1.0)
            ones_sq = constp.tile([P, P], BF16)
            nc.vector.tensor_copy(ones_sq, ones_sq_f)
            gate = constp.tile([P, 1], F32)
            nc.vector.memset(gate, 0.0)
            sc.update(ones_sq=ones_sq, gate=gate)

            res = {'qT': {}, 'kT': {}, 'v': {}}
            for b in range(c.B):
                res['qT'][b] = resp.tile([P, c.HPC, c.NB], BF16, name=f"qT_{b}")
                res['kT'][b] = resp.tile([P, c.HPC, c.NB], BF16, name=f"kT_{b}")
                res['v'][b] = resp.tile([P, c.KCH, c.G], BF16, name=f"v_{b}")
            io['res'] = res

            def body():
                _phase_a(nc, tc, c, io, sc, res)
                _phase_bc(nc, tc, c, io, sc)

            if c.loop_iters > 1:
                with tc.For_i(0, c.loop_iters, 1):
                    body()
            else:
                body()

    nc.compile()
    return nc


# ---------------------------------------------------------------------------
# host side
# ---------------------------------------------------------------------------

def rope_tables(T, H, W, head_dim):
    """cos/sin tables [T*H*W, head_dim], mirroring reference._rope_freqs."""
    dim_t = head_dim - 4 * (head_dim // 6)
    dim_h = 2 * (head_dim // 6)
    dim_w = 2 * (head_dim // 6)
    base = 10000.0
    ft = 1.0 / base ** (np.arange(0, dim_t, 2)[: dim_t // 2].astype(np.float32) / dim_t)
    fh = 1.0 / base ** (np.arange(0, dim_h, 2)[: dim_h // 2].astype(np.float32) / dim_h)
    fw = 1.0 / base ** (np.arange(0, dim_w, 2)[: dim_w // 2].astype(np.float32) / dim_w)
    gt = np.arange(T, dtype=np.float32)
    gh = np.arange(H, dtype=np.float32)
    gw = np.arange(W, dtype=np.float32)
    Ft = np.repeat(gt[:, None] * ft[None, :], 2, axis=-1)
    Fh = np.repeat(gh[:, None] * fh[None, :], 2, axis=-1)
    Fw = np.repeat(gw[:, None] * fw[None, :], 2, axis=-1)
    Ft = np.broadcast_to(Ft[:, None, None, :], (T, H, W, Ft.shape[-1]))
    Fh = np.broadcast_to(Fh[None, :, None, :], (T, H, W, Fh.shape[-1]))
    Fw = np.broadcast_to(Fw[None, None, :, :], (T, H, W, Fw.shape[-1]))
    freqs = np.concatenate([Ft, Fh, Fw], axis=-1).reshape(T * H * W, head_dim)
    return np.cos(freqs).astype(np.float32), np.sin(freqs).astype(np.float32)


def signed_sin(sin, w_for_pairs):
    """sinW[2i] = -sin[2i]*w[2i+1]; sinW[2i+1] = sin[2i+1]*w[2i]."""
    out = np.empty_like(sin)
    out[:, 0::2] = -sin[:, 0::2] * w_for_pairs[None, 1::2]
    out[:, 1::2] = sin[:, 1::2] * w_for_pairs[None, 0::2]
    return out


def make_in_maps(x, qkv_w, qkv_b, q_norm_w, k_norm_w, proj_w,
                 cos, sin, NB, CIN, COUT, HPC, B, D=128, ncores=8):
    import ml_dtypes
    bf = ml_dtypes.bfloat16
    TOK = B * NB
    Hn = ncores * HPC
    C_heads = Hn * D
    xT = x.reshape(TOK, CIN).T.astype(bf)
    KCn, TTn = CIN // D, TOK // D
    x2 = np.ascontiguousarray(
        xT.reshape(KCn, D, TTn, D).transpose(2, 1, 0, 3))
    def _retile(tab):
        tpb = tab.shape[0] // 128
        return np.ascontiguousarray(
            tab.reshape(tpb, 128, -1).transpose(1, 0, 2)).astype(np.float32)
    cosq = _retile(cos * q_norm_w[None, :])
    cosk = _retile(cos * k_norm_w[None, :])
    sinq = _retile(signed_sin(sin, q_norm_w))
    sink = _retile(signed_sin(sin, k_norm_w))
    in_maps = []
    for cix in range(ncores):
        G = HPC * D
        r0 = cix * G
        w_local = np.concatenate([
            qkv_w[r0:r0 + G],
            qkv_w[C_heads + r0:C_heads + r0 + G],
            qkv_w[2 * C_heads + r0:2 * C_heads + r0 + G],
        ], axis=0)
        wT_local = np.ascontiguousarray(w_local.T).astype(bf)
        b_local = np.concatenate([
            qkv_b[r0:r0 + G],
            qkv_b[C_heads + r0:C_heads + r0 + G],
        ]).astype(np.float32)
        pwT_local = np.ascontiguousarray(proj_w[:, r0:r0 + G].T).astype(bf)
        in_maps.append({
            "x2": x2, "wT": wT_local, "bias": b_local,
            "cosq": cosq, "sinq": sinq, "cosk": cosk, "sink": sink,
            "pwT": pwT_local,
        })
    return in_maps


# ---------------------------------------------------------------------------
# harness entry point
# ---------------------------------------------------------------------------

_CACHE = {}

_B, _NB, _CIN, _COUT, _D, _NCORES, _HPC = 2, 2048, 3072, 3072, 128, 8, 3


def _get_program(rope_shared):
    key = ("prog", rope_shared)
    if key not in _CACHE:
        _CACHE[key] = build_program(NB=_NB, CIN=_CIN, COUT=_COUT, HPC=_HPC,
                                    B=_B, D=_D, rope_shared=rope_shared)
    return _CACHE[key]


def kernel(x, qkv_w, qkv_b, q_norm_w, k_norm_w, proj_w, proj_b,
           t_size, h_size, w_size):
    from concourse import bass_utils

    x = np.asarray(x, dtype=np.float32)
    qkv_w = np.asarray(qkv_w, dtype=np.float32)
    qkv_b = np.asarray(qkv_b, dtype=np.float32)
    q_norm_w = np.asarray(q_norm_w, dtype=np.float32)
    k_norm_w = np.asarray(k_norm_w, dtype=np.float32)
    proj_w = np.asarray(proj_w, dtype=np.float32)
    proj_b = np.asarray(proj_b, dtype=np.float32)

    cos, sin = rope_tables(int(t_size), int(h_size), int(w_size), _D)
    rope_shared = (np.array_equal(q_norm_w, k_norm_w))
    nc = _get_program(rope_shared)

    in_maps = make_in_maps(x, qkv_w, qkv_b, q_norm_w, k_norm_w, proj_w,
                           cos, sin, _NB, _CIN, _COUT, _HPC, _B, _D, _NCORES)
    res = bass_utils.run_bass_kernel_spmd(
        nc, in_maps, core_ids=list(range(_NCORES)), trace=False)
    part = np.zeros((_B * _NB, _COUT), np.float64)
    for r in res.results:
        part += r["out_part"].astype(np.float64)
    bv = qkv_b[2 * _CIN:].astype(np.float64)
    const = bv @ proj_w.astype(np.float64).T + proj_b.astype(np.float64)
    out = (part + const).reshape(_B, _NB, _COUT)
    return out.astype(np.float32)
